# revision 8
# baseline (speedup 1.0000x reference)
"""Trainium2 Bass kernel for the dual-branch agent-attention module.

Sharding: data-parallel over B=8 (one batch element per NeuronCore).
All transposes and weight permutations are done host-side; on-device
work is a streamed bf16 pipeline:

  prep:      agent projections k_ag/qa -> block-diagonal tiles; then
             effective score weights Weff_A = Wq @ k12bd and
             Weff_B = Wkhf @ qabd (associativity: the big activations
             never materialize q or kh at all).
  phase B:   per seq tile, one weight-load of the attnT tile feeds
             three matmuls (v, scores[0:512], scores[512:768]); exp;
             xs accumulated directly in PSUM across all 32 seq tiles
             (softmax denom folded in via the ones column of v_aug).
  Z-prep:    xs normalized, transposed per head pair, and folded with
             Wproj into Z_j = xs_norm_j @ Wproj_j  [128 agent rows, C].
  phase AC:  sT scores from xT via Weff_A -> exp (P_u, agent-major);
             row sums D via indicator matmuls; 1/D broadcast back to
             agent rows via selector matmuls; P_norm = P_u * (1/D);
             out = sum_j P_norm_j^T @ Z_j + bproj.  No transposes, no
             separate x_out/proj stages.

Head-major layout trick: the 2C projection outputs are permuted host-
side from (branch, head, d) to (head, branch, d) so each head pair
occupies one 128-partition tile; branch score scales (wa/wb * D^-0.5)
are folded into the K-side weights.

Bias handling: k-side biases that are constant along a softmax axis
cancel exactly and are dropped (bk_hf entirely; q-side bias of branch
A survives as the per-agent term c_A = k12bd^T @ bq, applied as the
exp's per-partition bias together with ba).
"""

import os
import sys
import numpy as np

for _p in ("/opt/trn_rl_repo", os.path.expanduser("~/.axon_site/_ro/trn_rl_repo")):
    if os.path.isdir(_p) and _p not in sys.path:
        sys.path.insert(0, _p)

import ml_dtypes

import concourse.bass as bass
import concourse.bacc as bacc
import concourse.tile as tile
from concourse import mybir
from concourse.bass_utils import run_bass_kernel_spmd
from concourse.masks import make_identity

BF16 = mybir.dt.bfloat16
F32 = mybir.dt.float32
NPBF16 = ml_dtypes.bfloat16

B, N, NA, H, D = 8, 4096, 64, 12, 32
C = H * D            # 384
C2 = 2 * C           # 768
NP = H // 2          # 6 head pairs
CH = 512             # seq chunk
NCH = N // CH        # 8
TPC = CH // 128      # 4 seq tiles per chunk
NT = N // 128        # 32 seq tiles
SCALE = D ** -0.5

_CACHE = {}


def _build_bass(finalize=True, zero_bias=False):
    nc = bacc.Bacc()

    # ---- DRAM I/O ----
    xT = nc.dram_tensor("xT", [C, N], BF16, kind="ExternalInput")
    attnT = nc.dram_tensor("attnT", [C, N], BF16, kind="ExternalInput")
    agT = nc.dram_tensor("agT", [C, NA], BF16, kind="ExternalInput")
    wqT = nc.dram_tensor("wqT", [C2, C], BF16, kind="ExternalInput")
    wkag = nc.dram_tensor("wkag", [C, C2], BF16, kind="ExternalInput")
    wqag = nc.dram_tensor("wqag", [C, C2], BF16, kind="ExternalInput")
    wkhfT = nc.dram_tensor("wkhfT", [C2, C], BF16, kind="ExternalInput")
    wv = nc.dram_tensor("wv", [C, H * 33], BF16, kind="ExternalInput")
    wz = nc.dram_tensor("wz", [NP * 66, C], BF16, kind="ExternalInput")
    bq = nc.dram_tensor("bq", [C2], F32, kind="ExternalInput")
    bkag = nc.dram_tensor("bkag", [C2], F32, kind="ExternalInput")
    bqag = nc.dram_tensor("bqag", [C2], F32, kind="ExternalInput")
    bv = nc.dram_tensor("bv", [H * 33], F32, kind="ExternalInput")
    bproj = nc.dram_tensor("bproj", [C], F32, kind="ExternalInput")
    bab = nc.dram_tensor("bab", [2], F32, kind="ExternalInput")
    indc = nc.dram_tensor("indc", [128, NP * 12], BF16, kind="ExternalInput")
    selc = nc.dram_tensor("selc", [12, NP * 128], BF16, kind="ExternalInput")
    out = nc.dram_tensor("out", [N, C], F32, kind="ExternalOutput")

    Exp = mybir.ActivationFunctionType.Exp

    with tile.TileContext(nc) as tc:
        with (
            tc.tile_pool(name="const", bufs=1) as const,
            tc.tile_pool(name="inp", bufs=3) as p_in,
            tc.tile_pool(name="vsb", bufs=3) as p_v,
            tc.tile_pool(name="pt", bufs=3) as p_pt,
            tc.tile_pool(name="pa", bufs=2) as p_pa,
            tc.tile_pool(name="pn", bufs=2) as p_pn,
            tc.tile_pool(name="rd", bufs=2) as p_rd,
            tc.tile_pool(name="osb", bufs=3) as p_out,
            tc.tile_pool(name="sm", bufs=2) as p_sm,
            tc.tile_pool(name="ps", bufs=2, space="PSUM") as ps,
        ):
            # ---- constants (prep-critical DMAs first) ----
            ag_t = const.tile([128, 3, NA], BF16)
            nc.gpsimd.dma_start(out=ag_t, in_=agT.rearrange("(k p) m -> p k m", p=128))
            w_kag = const.tile([128, 3, C2], BF16)
            w_qag = const.tile([128, 3, C2], BF16)
            w_v = const.tile([128, 3, H * 33], BF16)
            w_qT = const.tile([128, 6, C], BF16)
            w_khfT = const.tile([128, 6, C], BF16)
            for dst, src in ((w_kag, wkag), (w_qag, wqag), (w_v, wv),
                             (w_khfT, wkhfT), (w_qT, wqT)):
                nc.gpsimd.dma_start(out=dst, in_=src.rearrange("(k p) m -> p k m", p=128))
            b_q = const.tile([128, 6], F32)
            b_kag = const.tile([128, 6], F32)
            b_qag = const.tile([128, 6], F32)
            for dst, src in ((b_q, bq), (b_kag, bkag), (b_qag, bqag)):
                nc.gpsimd.dma_start(out=dst, in_=src.rearrange("(j p) -> p j", p=128))
            bv_row = const.tile([1, H * 33], BF16)
            nc.gpsimd.dma_start(out=bv_row, in_=bv[:].unsqueeze(0))
            bpr_row = const.tile([1, C], BF16)
            nc.gpsimd.dma_start(out=bpr_row, in_=bproj[:].unsqueeze(0))
            w_z = const.tile([66, 6, C], BF16)
            nc.gpsimd.dma_start(out=w_z, in_=wz.rearrange("(j p) m -> p j m", p=66))
            ones_row = const.tile([1, 128], BF16)
            nc.vector.memset(ones_row, 1.0)
            ones12 = const.tile([1, 12], BF16)
            nc.vector.memset(ones12, 1.0)
            zeros396 = const.tile([1, 6 * 66], BF16)
            nc.vector.memset(zeros396, 0.0)
            ba_t = const.tile([128, 1], F32)
            nc.gpsimd.dma_start(out=ba_t, in_=bass.AP(tensor=bab[:].tensor, offset=0,
                                                      ap=[[0, 128], [1, 1]]))
            bb_t = const.tile([128, 1], F32)
            nc.gpsimd.dma_start(out=bb_t, in_=bass.AP(tensor=bab[:].tensor, offset=1,
                                                      ap=[[0, 128], [1, 1]]))
            ident = const.tile([128, 128], BF16)
            make_identity(nc, ident)
            # indicator: per pair j, col 2j = 1 on partitions 0:64 (head 2j
            # agents), col 2j+1 = 1 on partitions 64:128 (head 2j+1 agents);
            # selector: per pair j, row 2j -> partitions 0:64, row 2j+1 ->
            # 64:128.  Both host-prepared (unaligned-partition memsets are
            # not legal on DVE).
            ind = const.tile([128, 6, 12], BF16)
            nc.gpsimd.dma_start(out=ind, in_=indc.rearrange("p (j m) -> p j m", m=12))
            sel = const.tile([12, 6, 128], BF16)
            nc.gpsimd.dma_start(out=sel, in_=selc.rearrange("p (j m) -> p j m", m=128))

            # Pre-touch DMA-loaded constants with tiny reads so wide ops
            # downstream only carry the PE wait.
            touch = const.tile([128, 16], F32)
            for i, t_ap in enumerate((b_q[:, 0:1], b_kag[:, 0:1], b_qag[:, 0:1],
                                      ba_t[:, 0:1], bb_t[:, 0:1])):
                nc.vector.tensor_copy(touch[:, i:i + 1], t_ap)

            # ---- prep: k_ag / qa projections -> block-diag tiles ----
            kag_sb = const.tile([128, 6, NA], BF16)
            qa_sb = const.tile([128, 6, NA], BF16)
            for w_t, b_t, dst in ((w_kag, b_kag, kag_sb), (w_qag, b_qag, qa_sb)):
                for j in range(6):
                    pp = ps.tile([128, NA], F32, tag="v")
                    for k in range(3):
                        nc.tensor.matmul(pp, lhsT=w_t[:, k, j * 128:(j + 1) * 128],
                                         rhs=ag_t[:, k, :], start=(k == 0), stop=(k == 2))
                    nc.vector.tensor_add(dst[:, j, :], pp,
                                         b_t[:, j:j + 1].to_broadcast([128, NA]))
            k12bd = const.tile([128, 6, 128], BF16)
            qabd = const.tile([128, 6, 128], BF16)
            for src, dst in ((kag_sb, k12bd), (qa_sb, qabd)):
                nc.vector.memset(dst, 0.0)
                for j in range(6):
                    nc.vector.tensor_copy(dst[0:64, j, 0:64], src[0:64, j, :])
                    nc.vector.tensor_copy(dst[64:128, j, 64:128], src[64:128, j, :])

            # ---- prep: effective score weights + branch-A exp bias ----
            weff_a = const.tile([128, 3, C2], BF16)
            weff_b = const.tile([128, 3, C2], BF16)
            for j in range(6):
                for k in range(3):
                    pp = ps.tile([128, 128], F32, tag="s4")
                    nc.tensor.matmul(pp, lhsT=w_qT[:, j, k * 128:(k + 1) * 128],
                                     rhs=k12bd[:, j, :], start=True, stop=True)
                    nc.vector.tensor_copy(weff_a[:, k, j * 128:(j + 1) * 128], pp)
                    pp2 = ps.tile([128, 128], F32, tag="s2")
                    nc.tensor.matmul(pp2, lhsT=w_khfT[:, j, k * 128:(k + 1) * 128],
                                     rhs=qabd[:, j, :], start=True, stop=True)
                    nc.scalar.copy(weff_b[:, k, j * 128:(j + 1) * 128], pp2)
            cba = None
            if not zero_bias:
                b_q_bf = const.tile([128, 6], BF16)
                nc.vector.tensor_copy(b_q_bf, b_q)
                cba = const.tile([128, 6], F32)
                for j in range(6):
                    pp = ps.tile([128, 1], F32, tag="v")
                    nc.tensor.matmul(pp, lhsT=k12bd[:, j, :], rhs=b_q_bf[:, j:j + 1],
                                     start=True, stop=True)
                    nc.vector.tensor_add(cba[:, j:j + 1], pp, ba_t[:, 0:1])

            # ---- phase B: values + branch-B attention, xs in PSUM ----
            # xs accumulates across all 32 seq tiles in one PSUM bank.
            # First a zeroing matmul claims the bank (start=True sets
            # has_written for all 396 columns); the per-tile matmuls then
            # accumulate with start=False.
            xs_ps = ps.tile([128, 6 * 66], F32, tag="xs")
            nc.tensor.matmul(xs_ps, lhsT=ones_row, rhs=zeros396,
                             start=True, stop=False, skip_group_check=True)

            at_t = None
            stage = None  # (pt4, pt2, v_sb) of previous seq tile
            for i in range(NT + 1):
                if i < NT:
                    c, t = divmod(i, TPC)
                    if t == 0:
                        at_t = p_in.tile([128, 3, CH], BF16, tag="inp")
                        nc.sync.dma_start(
                            out=at_t,
                            in_=attnT.rearrange("(k p) s -> p k s", p=128)
                            [:, :, c * CH:(c + 1) * CH])
                    v_ps = ps.tile([128, H * 33], F32, tag="v")
                    s4 = ps.tile([128, 512], F32, tag="s4")
                    s2 = ps.tile([128, 256], F32, tag="s2")
                    for k in range(3):
                        lt = at_t[:, k, t * 128:(t + 1) * 128]
                        nc.tensor.matmul(v_ps, lhsT=lt, rhs=w_v[:, k, :],
                                         start=(k == 0), stop=False)
                        nc.tensor.matmul(s4, lhsT=lt, rhs=weff_b[:, k, 0:512],
                                         start=(k == 0), stop=(k == 2))
                        nc.tensor.matmul(s2, lhsT=lt, rhs=weff_b[:, k, 512:768],
                                         start=(k == 0), stop=(k == 2))
                    if zero_bias:
                        ps33 = v_ps.rearrange("p (h c) -> p h c", c=33)
                        nc.tensor.matmul(ps33[:, :, 32], lhsT=ones_row,
                                         rhs=ones12, start=False, stop=True)
                    else:
                        nc.tensor.matmul(v_ps, lhsT=ones_row, rhs=bv_row,
                                         start=False, stop=True)
                    bbias = 0.0 if zero_bias else bb_t[:, 0:1]
                    pt4 = p_pt.tile([128, 512], BF16, tag="pt4")
                    pt2 = p_pt.tile([128, 256], BF16, tag="pt2")
                    nc.scalar.activation(pt4, s4, Exp, bias=bbias)
                    nc.scalar.activation(pt2, s2, Exp, bias=bbias)
                    v_sb = p_v.tile([128, H * 33], BF16, tag="vsb")
                    nc.vector.tensor_copy(v_sb, v_ps)
                    nstage = (pt4, pt2, v_sb)
                if i >= 1:
                    q4, q2, vs = stage
                    last = (i - 1 == NT - 1)
                    for j in range(6):
                        lhsT = q4[:, j * 128:(j + 1) * 128] if j < 4 else \
                            q2[:, (j - 4) * 128:(j - 3) * 128]
                        nc.tensor.matmul(xs_ps[:, j * 66:(j + 1) * 66], lhsT=lhsT,
                                         rhs=vs[:, j * 66:(j + 1) * 66],
                                         start=False, stop=last,
                                         skip_group_check=True)
                if i < NT:
                    stage = nstage

            # ---- xs normalize -> block-diag tiles -> Z_j = xs_bd_j @ Wproj_j ----
            xs_bd = const.tile([128, 6, 66], BF16)
            xs3 = xs_ps.rearrange("p (j c) -> p j c", c=66)
            nc.vector.memset(xs_bd, 0.0)
            rec6 = p_sm.tile([128, 6], F32, tag="rec")
            nc.vector.reciprocal(rec6[0:64, :], xs3[0:64, :, 32])
            nc.vector.reciprocal(rec6[64:128, :], xs3[64:128, :, 65])
            nc.vector.tensor_mul(xs_bd[0:64, :, 0:32], xs3[0:64, :, 0:32],
                                 rec6[0:64, :].unsqueeze(2).to_broadcast([64, 6, 32]))
            nc.vector.tensor_mul(xs_bd[64:128, :, 33:65], xs3[64:128, :, 33:65],
                                 rec6[64:128, :].unsqueeze(2).to_broadcast([64, 6, 32]))
            xsT_sb = const.tile([66, 6, 128], BF16)
            z_sb = const.tile([128, 6, C], BF16)
            for j in range(6):
                tpz = ps.tile([66, 128], BF16, tag="v")
                nc.tensor.transpose(tpz, xs_bd[:, j, :], ident)
                nc.vector.tensor_copy(xsT_sb[:, j, :], tpz)
                z_ps = ps.tile([128, C], F32, tag="s4")
                nc.tensor.matmul(z_ps, lhsT=xsT_sb[:, j, :], rhs=w_z[:, j, :],
                                 start=True, stop=True)
                nc.vector.tensor_copy(z_sb[:, j, :], z_ps)

            # ---- phase AC: branch-A attention (agent-major) + fused out ----
            xt_t = None
            acst = None  # pn_sb of previous chunk
            for ci in range(NCH + 1):
                if ci < NCH:
                    xt_t = p_in.tile([128, 3, CH], BF16, tag="inp")
                    nc.sync.dma_start(
                        out=xt_t,
                        in_=xT.rearrange("(k p) s -> p k s", p=128)
                        [:, :, ci * CH:(ci + 1) * CH])
                    pa_sb = p_pa.tile([128, 6, CH], BF16, tag="pa")
                    d_ps = ps.tile([12, 512], F32, tag="s2")
                    for j in range(6):
                        sa = ps.tile([128, CH], F32, tag="s4")
                        for k in range(3):
                            nc.tensor.matmul(sa, lhsT=weff_a[:, k, j * 128:(j + 1) * 128],
                                             rhs=xt_t[:, k, :], start=(k == 0),
                                             stop=(k == 2))
                        nc.scalar.activation(pa_sb[:, j, :], sa, Exp,
                                             bias=(0.0 if zero_bias else cba[:, j:j + 1]))
                        nc.tensor.matmul(d_ps, lhsT=ind[:, j, :], rhs=pa_sb[:, j, :],
                                         start=(j == 0), stop=(j == 5))
                # out matmuls of the previous chunk fill the PE while this
                # chunk's exp/recip run on scalar/vector.
                if ci >= 1:
                    pn_prev = acst
                    for t in range(TPC):
                        o_ps = ps.tile([128, C], F32, tag="xs")
                        for j in range(6):
                            nc.tensor.matmul(o_ps, lhsT=pn_prev[:, j, t * 128:(t + 1) * 128],
                                             rhs=z_sb[:, j, :], start=(j == 0),
                                             stop=(j == 5 and zero_bias))
                        if not zero_bias:
                            nc.tensor.matmul(o_ps, lhsT=ones_row, rhs=bpr_row,
                                             start=False, stop=True)
                        o_sb = p_out.tile([128, C], F32, tag="osb")
                        if t % 2 == 0:
                            nc.scalar.copy(o_sb, o_ps)
                        else:
                            nc.vector.tensor_copy(o_sb, o_ps)
                        r0 = ((ci - 1) * TPC + t) * 128
                        nc.sync.dma_start(out=out[r0:r0 + 128, :], in_=o_sb)
                if ci < NCH:
                    rd_sb = p_rd.tile([12, 512], BF16, tag="rd")
                    with nc.allow_low_precision(reason="bf16 softmax recip matches baseline xon rounding"):
                        nc.vector.reciprocal(rd_sb, d_ps)
                    pn_sb = p_pn.tile([128, 6, CH], BF16, tag="pn")
                    for j in range(6):
                        bc = ps.tile([128, 512], F32, tag="v")
                        nc.tensor.matmul(bc, lhsT=sel[:, j, :], rhs=rd_sb,
                                         start=True, stop=True)
                        nc.vector.tensor_mul(pn_sb[:, j, :], pa_sb[:, j, :], bc)
                    acst = pn_sb
    if finalize:
        nc.finalize()
    return nc


def _prep_host(inputs):
    f32 = np.float32
    x = np.asarray(inputs["x"], f32)
    attn = np.asarray(inputs["attn"], f32)
    agent = np.asarray(inputs["agent_input"], f32)
    wa = np.asarray(inputs["wa"], f32)
    wb = np.asarray(inputs["wb"], f32)

    perm = np.empty(C2, np.int64)
    sva = np.empty(C2, f32)
    svb = np.empty(C2, f32)
    for h in range(H):
        for br in range(2):
            j0 = h * 64 + br * 32
            perm[j0:j0 + 32] = br * C + h * 32 + np.arange(32)
            sva[j0:j0 + 32] = wa[br] * SCALE
            svb[j0:j0 + 32] = wb[br] * SCALE

    wq_p = np.asarray(inputs["Wq_lf"], f32)[:, perm]
    bq_p = np.asarray(inputs["bq_lf"], f32)[perm]
    wkag_p = np.asarray(inputs["Wk_ag"], f32)[:, perm] * sva[None, :]
    bkag_p = np.asarray(inputs["bk_ag"], f32)[perm] * sva
    wqag_p = np.asarray(inputs["Wq_ag"], f32)[:, perm]
    bqag_p = np.asarray(inputs["bq_ag"], f32)[perm]
    wkhf_p = np.asarray(inputs["Wk_hf"], f32)[:, perm] * svb[None, :]

    wv_in = np.asarray(inputs["Wv_hf"], f32)
    bv_in = np.asarray(inputs["bv_hf"], f32)
    wv_aug = np.zeros((C, H * 33), f32)
    bv_aug = np.zeros(H * 33, f32)
    for h in range(H):
        wv_aug[:, h * 33:h * 33 + 32] = wv_in[:, h * 32:h * 32 + 32]
        bv_aug[h * 33:h * 33 + 32] = bv_in[h * 32:h * 32 + 32]
        bv_aug[h * 33 + 32] = 1.0

    # Z-side projection: rows of Wproj regrouped per head pair with zero
    # rows at the ones-column positions of xs_bd (33-row blocks per head).
    wproj_in = np.asarray(inputs["Wproj"], f32)
    wz_aug = np.zeros((NP * 66, C), f32)
    for j in range(NP):
        wz_aug[j * 66:j * 66 + 32, :] = wproj_in[(2 * j) * 32:(2 * j + 1) * 32, :]
        wz_aug[j * 66 + 33:j * 66 + 65, :] = wproj_in[(2 * j + 1) * 32:(2 * j + 2) * 32, :]

    bab = np.array([np.asarray(inputs["ba"], f32)[0],
                    np.asarray(inputs["bb"], f32)[0]], f32)

    ind_c = np.zeros((128, NP, 12), f32)
    sel_c = np.zeros((12, NP, 128), f32)
    for j in range(NP):
        ind_c[0:64, j, 2 * j] = 1.0
        ind_c[64:128, j, 2 * j + 1] = 1.0
        sel_c[2 * j, j, 0:64] = 1.0
        sel_c[2 * j + 1, j, 64:128] = 1.0

    shared = {
        "wqT": np.ascontiguousarray(wq_p.T).astype(NPBF16),
        "wkhfT": np.ascontiguousarray(wkhf_p.T).astype(NPBF16),
        "wkag": wkag_p.astype(NPBF16),
        "wqag": wqag_p.astype(NPBF16),
        "wv": wv_aug.astype(NPBF16),
        "wz": wz_aug.astype(NPBF16),
        "indc": ind_c.reshape(128, NP * 12).astype(NPBF16),
        "selc": sel_c.reshape(12, NP * 128).astype(NPBF16),
        "bq": bq_p, "bkag": bkag_p, "bqag": bqag_p,
        "bv": bv_aug, "bproj": np.ascontiguousarray(np.asarray(inputs["bproj"], f32)),
        "bab": bab,
    }
    xT = np.ascontiguousarray(x.transpose(0, 2, 1)).astype(NPBF16)
    attnT = np.ascontiguousarray(attn.transpose(0, 2, 1)).astype(NPBF16)
    agT = np.ascontiguousarray(agent.transpose(0, 2, 1)).astype(NPBF16)
    in_maps = []
    for b in range(B):
        m = dict(shared)
        m["xT"] = xT[b]
        m["attnT"] = attnT[b]
        m["agT"] = agT[b]
        in_maps.append(m)
    return in_maps


def kernel(**inputs):
    zb = all(not np.any(np.asarray(inputs[k]))
             for k in ("bq_lf", "bk_ag", "bq_ag", "bk_hf", "bv_hf", "bproj",
                       "ba", "bb"))
    key = ("nc", zb)
    if key not in _CACHE:
        _CACHE[key] = _build_bass(zero_bias=zb)
    nc = _CACHE[key]
    in_maps = _prep_host(inputs)
    res = run_bass_kernel_spmd(nc, in_maps, core_ids=list(range(B)))
    return np.stack([res.results[b]["out"] for b in range(B)], axis=0)


# revision 9
# speedup vs baseline: 1.4940x; 1.4940x over previous
"""Trainium2 Bass kernel for the dual-branch agent-attention module.

Sharding: data-parallel over B=8 (one batch element per NeuronCore).
All transposes and weight permutations are done host-side; on-device
work is a streamed pipeline:

  prep:      agent projections k_ag/qa -> block-diagonal tiles; then
             effective score weights Weff_A = Wq @ k12bd and
             Weff_B = Wkhf @ qabd (associativity: the big activations
             never materialize q or kh at all), cast to fp8 with a
             x128 scale (undone in the exp's input scale).
  phase B:   per seq tile: v = attnT^T @ Wv in bf16; branch-B scores
             directly from fp8 attnT via fp8 Weff_B in DoubleRow mode
             (256-deep contraction per pass, 2 passes over the padded
             512 channels); exp; xs accumulated in PSUM across all 32
             seq tiles (softmax denom via the ones column of v_aug).
  phase AC:  branch-A scores from fp8 xT via fp8 Weff_A (DoubleRow),
             exp -> PA, x_out with ones-column denom, normalize,
             PE-transpose, proj.

Scores tolerate fp8 because the logits are small (near-uniform
softmax): a ~6% relative weight-quantization error moves attention
weights by ~1%.  v and proj stay bf16 since their errors flow
linearly to the output.

Bias handling: k-side biases that are constant along a softmax axis
cancel exactly and are dropped (bk_hf entirely; q-side bias of branch
A survives as the per-agent term c_A = k12bd^T @ bq, applied as the
exp's per-partition bias together with ba).
"""

import os
import sys
import numpy as np

for _p in ("/opt/trn_rl_repo", os.path.expanduser("~/.axon_site/_ro/trn_rl_repo")):
    if os.path.isdir(_p) and _p not in sys.path:
        sys.path.insert(0, _p)

import ml_dtypes

import concourse.bass as bass
import concourse.bacc as bacc
import concourse.tile as tile
from concourse import mybir
from concourse.bass_utils import run_bass_kernel_spmd
from concourse.masks import make_identity

BF16 = mybir.dt.bfloat16
FP8 = mybir.dt.float8e4
F32 = mybir.dt.float32
NPBF16 = ml_dtypes.bfloat16
NPFP8 = ml_dtypes.float8_e4m3

B, N, NA, H, D = 8, 4096, 64, 12, 32
C = H * D            # 384
CP = 512             # fp8-padded channel count (4 x 128 k-tiles)
C2 = 2 * C           # 768
NP = H // 2          # 6 head pairs
CH = 512             # seq chunk
NCH = N // CH        # 8
TPC = CH // 128      # 4 seq tiles per chunk
NT = N // 128        # 32 seq tiles
SCALE = D ** -0.5
FS = 128.0           # fp8 weight scale (undone by exp input scale)
DR = mybir.MatmulPerfMode.DoubleRow

_CACHE = {}


def _build_bass(finalize=True, zero_bias=False):
    nc = bacc.Bacc()

    # ---- DRAM I/O ----
    attnT = nc.dram_tensor("attnT", [C, N], BF16, kind="ExternalInput")
    attnT8 = nc.dram_tensor("attnT8", [CP, N], FP8, kind="ExternalInput")
    xT8 = nc.dram_tensor("xT8", [CP, N], FP8, kind="ExternalInput")
    agT = nc.dram_tensor("agT", [C, NA], BF16, kind="ExternalInput")
    wqT = nc.dram_tensor("wqT", [C2, C], BF16, kind="ExternalInput")
    wkag = nc.dram_tensor("wkag", [C, C2], BF16, kind="ExternalInput")
    wqag = nc.dram_tensor("wqag", [C, C2], BF16, kind="ExternalInput")
    wkhfT = nc.dram_tensor("wkhfT", [C2, C], BF16, kind="ExternalInput")
    wv = nc.dram_tensor("wv", [C, H * 33], BF16, kind="ExternalInput")
    wproj = nc.dram_tensor("wproj", [C, C], BF16, kind="ExternalInput")
    bq = nc.dram_tensor("bq", [C2], F32, kind="ExternalInput")
    bkag = nc.dram_tensor("bkag", [C2], F32, kind="ExternalInput")
    bqag = nc.dram_tensor("bqag", [C2], F32, kind="ExternalInput")
    bv = nc.dram_tensor("bv", [H * 33], F32, kind="ExternalInput")
    bproj = nc.dram_tensor("bproj", [C], F32, kind="ExternalInput")
    bab = nc.dram_tensor("bab", [2], F32, kind="ExternalInput")
    out = nc.dram_tensor("out", [N, C], F32, kind="ExternalOutput")

    Exp = mybir.ActivationFunctionType.Exp
    Copy = mybir.ActivationFunctionType.Copy
    INV_FS = 1.0 / FS

    with tile.TileContext(nc) as tc:
        with (
            tc.tile_pool(name="const", bufs=1) as const,
            tc.tile_pool(name="inp", bufs=3) as p_in,
            tc.tile_pool(name="vsb", bufs=3) as p_v,
            tc.tile_pool(name="pt", bufs=3) as p_pt,
            tc.tile_pool(name="pa", bufs=2) as p_pa,
            tc.tile_pool(name="xon", bufs=3) as p_xon,
            tc.tile_pool(name="xot", bufs=3) as p_xot,
            tc.tile_pool(name="osb", bufs=3) as p_out,
            tc.tile_pool(name="sm", bufs=4) as p_sm,
            tc.tile_pool(name="ps", bufs=2, space="PSUM") as ps,
        ):
            # ---- constants (prep-critical DMAs first) ----
            ag_t = const.tile([128, 3, NA], BF16)
            nc.gpsimd.dma_start(out=ag_t, in_=agT.rearrange("(k p) m -> p k m", p=128))
            w_kag = const.tile([128, 3, C2], BF16)
            w_qag = const.tile([128, 3, C2], BF16)
            w_v = const.tile([128, 3, H * 33], BF16)
            w_qT = const.tile([128, 6, C], BF16)
            w_khfT = const.tile([128, 6, C], BF16)
            for dst, src in ((w_kag, wkag), (w_qag, wqag), (w_v, wv),
                             (w_khfT, wkhfT), (w_qT, wqT)):
                nc.gpsimd.dma_start(out=dst, in_=src.rearrange("(k p) m -> p k m", p=128))
            b_q = const.tile([128, 6], F32)
            b_kag = const.tile([128, 6], F32)
            b_qag = const.tile([128, 6], F32)
            for dst, src in ((b_q, bq), (b_kag, bkag), (b_qag, bqag)):
                nc.gpsimd.dma_start(out=dst, in_=src.rearrange("(j p) -> p j", p=128))
            bv_row = const.tile([1, H * 33], BF16)
            nc.gpsimd.dma_start(out=bv_row, in_=bv[:].unsqueeze(0))
            bpr_row = const.tile([1, C], BF16)
            nc.gpsimd.dma_start(out=bpr_row, in_=bproj[:].unsqueeze(0))
            ones_row = const.tile([1, 128], BF16)
            nc.vector.memset(ones_row, 1.0)
            ones12 = const.tile([1, 12], BF16)
            nc.vector.memset(ones12, 1.0)
            zeros396 = const.tile([1, 6 * 66], BF16)
            nc.vector.memset(zeros396, 0.0)
            ba_t = const.tile([128, 1], F32)
            nc.gpsimd.dma_start(out=ba_t, in_=bass.AP(tensor=bab[:].tensor, offset=0,
                                                      ap=[[0, 128], [1, 1]]))
            bb_t = const.tile([128, 1], F32)
            nc.gpsimd.dma_start(out=bb_t, in_=bass.AP(tensor=bab[:].tensor, offset=1,
                                                      ap=[[0, 128], [1, 1]]))
            ident = const.tile([128, 128], BF16)
            make_identity(nc, ident)

            # Pre-touch DMA-loaded constants with tiny reads so wide ops
            # downstream only carry the PE wait.
            touch = const.tile([128, 16], F32)
            for i, t_ap in enumerate((b_q[:, 0:1], b_kag[:, 0:1], b_qag[:, 0:1],
                                      ba_t[:, 0:1], bb_t[:, 0:1])):
                nc.vector.tensor_copy(touch[:, i:i + 1], t_ap)

            # ---- prep: k_ag / qa projections -> block-diag tiles ----
            kag_sb = const.tile([128, 6, NA], BF16)
            qa_sb = const.tile([128, 6, NA], BF16)
            for w_t, b_t, dst in ((w_kag, b_kag, kag_sb), (w_qag, b_qag, qa_sb)):
                for j in range(6):
                    pp = ps.tile([128, NA], F32, tag="v")
                    for k in range(3):
                        nc.tensor.matmul(pp, lhsT=w_t[:, k, j * 128:(j + 1) * 128],
                                         rhs=ag_t[:, k, :], start=(k == 0), stop=(k == 2))
                    nc.vector.tensor_add(dst[:, j, :], pp,
                                         b_t[:, j:j + 1].to_broadcast([128, NA]))
            k12bd = const.tile([128, 6, 128], BF16)
            qabd = const.tile([128, 6, 128], BF16)
            for src, dst in ((kag_sb, k12bd), (qa_sb, qabd)):
                nc.vector.memset(dst, 0.0)
                for j in range(6):
                    nc.vector.tensor_copy(dst[0:64, j, 0:64], src[0:64, j, :])
                    nc.vector.tensor_copy(dst[64:128, j, 64:128], src[64:128, j, :])

            # ---- prep: effective score weights (fp8, x128) + exp bias ----
            weff_a8 = const.tile([128, 4, C2], FP8)
            weff_b8 = const.tile([128, 4, C2], FP8)
            nc.vector.memset(weff_a8, 0.0)
            nc.vector.memset(weff_b8, 0.0)
            for j in range(6):
                for k in range(3):
                    pp = ps.tile([128, 128], F32, tag="s4")
                    nc.tensor.matmul(pp, lhsT=w_qT[:, j, k * 128:(k + 1) * 128],
                                     rhs=k12bd[:, j, :], start=True, stop=True)
                    nc.scalar.activation(weff_a8[:, k, j * 128:(j + 1) * 128], pp,
                                         Copy, bias=0.0, scale=FS)
                    pp2 = ps.tile([128, 128], F32, tag="s2")
                    nc.tensor.matmul(pp2, lhsT=w_khfT[:, j, k * 128:(k + 1) * 128],
                                     rhs=qabd[:, j, :], start=True, stop=True)
                    nc.scalar.activation(weff_b8[:, k, j * 128:(j + 1) * 128], pp2,
                                         Copy, bias=0.0, scale=FS)
            cba = None
            if not zero_bias:
                b_q_bf = const.tile([128, 6], BF16)
                nc.vector.tensor_copy(b_q_bf, b_q)
                cba = const.tile([128, 6], F32)
                for j in range(6):
                    pp = ps.tile([128, 1], F32, tag="v")
                    nc.tensor.matmul(pp, lhsT=k12bd[:, j, :], rhs=b_q_bf[:, j:j + 1],
                                     start=True, stop=True)
                    nc.vector.tensor_add(cba[:, j:j + 1], pp, ba_t[:, 0:1])

            # ---- phase B: values + branch-B attention, xs in PSUM ----
            # xs accumulates across all 32 seq tiles in one PSUM bank.  A
            # zeroing matmul claims the bank (start=True sets has_written
            # for all 396 columns); per-tile matmuls accumulate after it.
            xs_ps = ps.tile([128, 6 * 66], F32, tag="xs")
            nc.tensor.matmul(xs_ps, lhsT=ones_row, rhs=zeros396,
                             start=True, stop=False, skip_group_check=True)

            at_t = at8_t = None
            stage = None  # (pt4, pt2, v_sb) of previous seq tile
            for i in range(NT + 1):
                if i < NT:
                    c, t = divmod(i, TPC)
                    if t == 0:
                        at_t = p_in.tile([128, 3, CH], BF16, tag="inp")
                        nc.sync.dma_start(
                            out=at_t,
                            in_=attnT.rearrange("(k p) s -> p k s", p=128)
                            [:, :, c * CH:(c + 1) * CH])
                        at8_t = p_in.tile([128, 4, CH], FP8, tag="inp8")
                        nc.sync.dma_start(
                            out=at8_t,
                            in_=attnT8.rearrange("(k p) s -> p k s", p=128)
                            [:, :, c * CH:(c + 1) * CH])
                    v_ps = ps.tile([128, H * 33], F32, tag="v")
                    s4 = ps.tile([128, 512], F32, tag="s4")
                    s2 = ps.tile([128, 256], F32, tag="s2")
                    for k in range(3):
                        nc.tensor.matmul(v_ps, lhsT=at_t[:, k, t * 128:(t + 1) * 128],
                                         rhs=w_v[:, k, :], start=(k == 0), stop=False)
                    if zero_bias:
                        ps33 = v_ps.rearrange("p (h c) -> p h c", c=33)
                        nc.tensor.matmul(ps33[:, :, 32], lhsT=ones_row,
                                         rhs=ones12, start=False, stop=True)
                    else:
                        nc.tensor.matmul(v_ps, lhsT=ones_row, rhs=bv_row,
                                         start=False, stop=True)
                    for p in range(2):
                        lt8 = at8_t[:, 2 * p:2 * p + 2, t * 128:(t + 1) * 128]
                        nc.tensor.matmul(s4, lhsT=lt8,
                                         rhs=weff_b8[:, 2 * p:2 * p + 2, 0:512],
                                         start=(p == 0), stop=(p == 1), perf_mode=DR)
                        nc.tensor.matmul(s2, lhsT=lt8,
                                         rhs=weff_b8[:, 2 * p:2 * p + 2, 512:768],
                                         start=(p == 0), stop=(p == 1), perf_mode=DR)
                    bbias = 0.0 if zero_bias else bb_t[:, 0:1]
                    pt4 = p_pt.tile([128, 512], BF16, tag="pt4")
                    pt2 = p_pt.tile([128, 256], BF16, tag="pt2")
                    nc.scalar.activation(pt4, s4, Exp, bias=bbias, scale=INV_FS)
                    nc.scalar.activation(pt2, s2, Exp, bias=bbias, scale=INV_FS)
                    v_sb = p_v.tile([128, H * 33], BF16, tag="vsb")
                    nc.vector.tensor_copy(v_sb, v_ps)
                    nstage = (pt4, pt2, v_sb)
                if i >= 1:
                    q4, q2, vs = stage
                    last = (i - 1 == NT - 1)
                    for j in range(6):
                        lhsT = q4[:, j * 128:(j + 1) * 128] if j < 4 else \
                            q2[:, (j - 4) * 128:(j - 3) * 128]
                        nc.tensor.matmul(xs_ps[:, j * 66:(j + 1) * 66], lhsT=lhsT,
                                         rhs=vs[:, j * 66:(j + 1) * 66],
                                         start=False, stop=last,
                                         skip_group_check=True)
                if i < NT:
                    stage = nstage

            w_pr = const.tile([128, 3, C], BF16)
            nc.gpsimd.dma_start(out=w_pr, in_=wproj.rearrange("(k p) m -> p k m", p=128))

            # ---- xs normalize -> block-diag [xs | 1] tiles ----
            xs_bd = const.tile([128, 6, 66], BF16)
            xs3 = xs_ps.rearrange("p (j c) -> p j c", c=66)
            bd3 = xs_bd
            nc.vector.memset(xs_bd, 0.0)
            nc.vector.memset(bd3[0:64, :, 32:33], 1.0)
            nc.vector.memset(bd3[64:128, :, 65:66], 1.0)
            rec6 = p_sm.tile([128, 6], F32, tag="rec")
            nc.vector.reciprocal(rec6[0:64, :], xs3[0:64, :, 32])
            nc.vector.reciprocal(rec6[64:128, :], xs3[64:128, :, 65])
            nc.vector.tensor_mul(bd3[0:64, :, 0:32], xs3[0:64, :, 0:32],
                                 rec6[0:64, :].unsqueeze(2).to_broadcast([64, 6, 32]))
            nc.vector.tensor_mul(bd3[64:128, :, 33:65], xs3[64:128, :, 33:65],
                                 rec6[64:128, :].unsqueeze(2).to_broadcast([64, 6, 32]))

            # ---- phase AC: branch-A attention + proj ----
            for ci in range(NCH):
                xt8_t = p_in.tile([128, 4, CH], FP8, tag="inp8")
                nc.sync.dma_start(
                    out=xt8_t,
                    in_=xT8.rearrange("(k p) s -> p k s", p=128)
                    [:, :, ci * CH:(ci + 1) * CH])
                pa_t = p_pa.tile([128, 6, CH], BF16, tag="pa")
                for j in range(6):
                    sa = ps.tile([128, CH], F32, tag="s4")
                    for p in range(2):
                        nc.tensor.matmul(sa,
                                         lhsT=weff_a8[:, 2 * p:2 * p + 2,
                                                      j * 128:(j + 1) * 128],
                                         rhs=xt8_t[:, 2 * p:2 * p + 2, :],
                                         start=(p == 0), stop=(p == 1), perf_mode=DR)
                    nc.scalar.activation(pa_t[:, j, :], sa, Exp,
                                         bias=(0.0 if zero_bias else cba[:, j:j + 1]),
                                         scale=INV_FS)
                for t in range(TPC):
                    xo_ps = ps.tile([128, 12 * 33], F32, tag="v")
                    for j in range(6):
                        nc.tensor.matmul(xo_ps[:, j * 66:(j + 1) * 66],
                                         lhsT=pa_t[:, j, t * 128:(t + 1) * 128],
                                         rhs=xs_bd[:, j, :],
                                         start=True, stop=True)
                    xo3 = xo_ps.rearrange("p (k c) -> p k c", c=33)
                    rec = p_sm.tile([128, 12], F32, tag="rec12")
                    nc.vector.reciprocal(rec, xo3[:, :, 32])
                    xon = p_xon.tile([128, C], BF16)
                    nc.vector.tensor_mul(xon.rearrange("p (k c) -> p k c", c=32),
                                         xo3[:, :, 0:32],
                                         rec.unsqueeze(2).to_broadcast([128, 12, 32]))
                    pr_ps = ps.tile([128, C], F32, tag="xs")
                    for f in range(3):
                        tp = ps.tile([128, 128], BF16, tag="s2")
                        nc.tensor.transpose(tp, xon[:, f * 128:(f + 1) * 128], ident)
                        xot = p_xot.tile([128, 128], BF16)
                        nc.vector.tensor_copy(xot, tp)
                        nc.tensor.matmul(pr_ps, lhsT=xot, rhs=w_pr[:, f, :],
                                         start=(f == 0),
                                         stop=(zero_bias and f == 2),
                                         skip_group_check=True)
                    if not zero_bias:
                        nc.tensor.matmul(pr_ps, lhsT=ones_row, rhs=bpr_row,
                                         start=False, stop=True, skip_group_check=True)
                    o_sb = p_out.tile([128, C], F32)
                    if t % 2 == 0:
                        nc.scalar.copy(o_sb, pr_ps)
                    else:
                        nc.vector.tensor_copy(o_sb, pr_ps)
                    r0 = (ci * TPC + t) * 128
                    nc.sync.dma_start(out=out[r0:r0 + 128, :], in_=o_sb)
    if finalize:
        nc.finalize()
    return nc


def _prep_host(inputs):
    f32 = np.float32
    x = np.asarray(inputs["x"], f32)
    attn = np.asarray(inputs["attn"], f32)
    agent = np.asarray(inputs["agent_input"], f32)
    wa = np.asarray(inputs["wa"], f32)
    wb = np.asarray(inputs["wb"], f32)

    perm = np.empty(C2, np.int64)
    sva = np.empty(C2, f32)
    svb = np.empty(C2, f32)
    for h in range(H):
        for br in range(2):
            j0 = h * 64 + br * 32
            perm[j0:j0 + 32] = br * C + h * 32 + np.arange(32)
            sva[j0:j0 + 32] = wa[br] * SCALE
            svb[j0:j0 + 32] = wb[br] * SCALE

    wq_p = np.asarray(inputs["Wq_lf"], f32)[:, perm]
    bq_p = np.asarray(inputs["bq_lf"], f32)[perm]
    wkag_p = np.asarray(inputs["Wk_ag"], f32)[:, perm] * sva[None, :]
    bkag_p = np.asarray(inputs["bk_ag"], f32)[perm] * sva
    wqag_p = np.asarray(inputs["Wq_ag"], f32)[:, perm]
    bqag_p = np.asarray(inputs["bq_ag"], f32)[perm]
    wkhf_p = np.asarray(inputs["Wk_hf"], f32)[:, perm] * svb[None, :]

    wv_in = np.asarray(inputs["Wv_hf"], f32)
    bv_in = np.asarray(inputs["bv_hf"], f32)
    wv_aug = np.zeros((C, H * 33), f32)
    bv_aug = np.zeros(H * 33, f32)
    for h in range(H):
        wv_aug[:, h * 33:h * 33 + 32] = wv_in[:, h * 32:h * 32 + 32]
        bv_aug[h * 33:h * 33 + 32] = bv_in[h * 32:h * 32 + 32]
        bv_aug[h * 33 + 32] = 1.0

    bab = np.array([np.asarray(inputs["ba"], f32)[0],
                    np.asarray(inputs["bb"], f32)[0]], f32)

    shared = {
        "wqT": np.ascontiguousarray(wq_p.T).astype(NPBF16),
        "wkhfT": np.ascontiguousarray(wkhf_p.T).astype(NPBF16),
        "wkag": wkag_p.astype(NPBF16),
        "wqag": wqag_p.astype(NPBF16),
        "wv": wv_aug.astype(NPBF16),
        "wproj": np.asarray(inputs["Wproj"], f32).astype(NPBF16),
        "bq": bq_p, "bkag": bkag_p, "bqag": bqag_p,
        "bv": bv_aug, "bproj": np.ascontiguousarray(np.asarray(inputs["bproj"], f32)),
        "bab": bab,
    }
    attnT = np.ascontiguousarray(attn.transpose(0, 2, 1)).astype(NPBF16)
    # fp8 copies, channel-padded to 512 for DoubleRow's 2x256 contraction
    xT8 = np.zeros((B, CP, N), NPFP8)
    attnT8 = np.zeros((B, CP, N), NPFP8)
    xT8[:, :C, :] = np.clip(x.transpose(0, 2, 1), -240, 240).astype(NPFP8)
    attnT8[:, :C, :] = np.clip(attn.transpose(0, 2, 1), -240, 240).astype(NPFP8)
    agT = np.ascontiguousarray(agent.transpose(0, 2, 1)).astype(NPBF16)
    in_maps = []
    for b in range(B):
        m = dict(shared)
        m["attnT"] = attnT[b]
        m["attnT8"] = attnT8[b]
        m["xT8"] = xT8[b]
        m["agT"] = agT[b]
        in_maps.append(m)
    return in_maps


def kernel(**inputs):
    zb = all(not np.any(np.asarray(inputs[k]))
             for k in ("bq_lf", "bk_ag", "bq_ag", "bk_hf", "bv_hf", "bproj",
                       "ba", "bb"))
    key = ("nc", zb)
    if key not in _CACHE:
        _CACHE[key] = _build_bass(zero_bias=zb)
    nc = _CACHE[key]
    in_maps = _prep_host(inputs)
    res = run_bass_kernel_spmd(nc, in_maps, core_ids=list(range(B)))
    return np.stack([res.results[b]["out"] for b in range(B)], axis=0)
